# revision 3
# baseline (speedup 1.0000x reference)
"""Distributed Trainium2 kernel for AttHGCNConv:
out = LeakyReLU_0.2( A @ B @ (B.T @ (A.T @ embs)) ),  A=att_adj [N,E], B=inp_adj [E,N].

Never materializes adj = A@B (~1.1 TFLOP); chains 4 thin matmuls of 34 GFLOP
each — memory-bound. 8-way sharded, ALL-GATHER ONLY dataflow (AllGather on
8 cores is ~5-14us vs ~55-68us for AllReduce, since AR = RS+AG with 2x the
M2S descriptor traffic):
  S1 (local): t1_c = A[:,e_c].T @ embs        lhsT = a_g   (A col-shard)
  AG(t1): 0.5MB/rank -> t1 full [E,D]
  S2 (local): t2[n_c] = B[:,n_c].T @ t1       lhsT = b2_g  (B col-shard)
     full 64-tile contraction in PSUM f32 — no partial-sum wire rounding
  AG(t2): -> t2 full [N,D]
  S3 (local): t3_c = B[e_c,:] @ t2            lhsT = bt_g  (B row-shard^T)
  AG(t3): -> t3 full [E,D]
  S4 (local): out[n_c] = A[n_c,:] @ t3        lhsT = a2_g  (A row-shard^T)
     LeakyReLU fused in the PSUM->SBUF eviction; f32 rows stored directly.
     NO final collective, no fp16-range rescaling dance.

fp16 operands (PSUM accumulates f32), fp16 collective wires. Collective
bounce buffers are declared partition-major [128, 8*256] so every wire-facing
DMA is a single linear 0.5MB transfer. All weights host-relaid so every
weight DMA is a single 1MB linear read.
"""

import sys

for p in ("/opt/trn_rl_repo", "/root/.axon_site"):
    if p not in sys.path:
        sys.path.insert(0, p)

import numpy as np

import concourse.bass as bass  # noqa: F401
import concourse.mybir as mybir
import concourse.tile as tile
from concourse import bacc
from concourse.bass_utils import run_bass_kernel_spmd

N_CORES = 8
N = 8192  # nodes
E = 8192  # hyperedges
D = 256   # embedding dim
S = E // N_CORES   # 1024 per-core shard
KT = 128           # partition tile
NK = N // KT       # 64
SK = S // KT       # 8
LEAKY = 0.2

BW_ = 4                      # k-tiles fused per weight DMA (1MB each)
NG = NK // BW_               # 16 weight DMAs per matrix
EB = 16                      # embs k-tiles per DMA

W16 = mybir.dt.float16       # matmul operand / wire dtype
F32 = mybir.dt.float32
NP16 = np.float16

_CACHED_NC = None


def _build():
    nc = bacc.Bacc("TRN2", target_bir_lowering=False, debug=False,
                   num_devices=N_CORES)

    # weights pre-fused on host: [NG, 128, BW_*1024], one 1MB DMA per row
    a_g = nc.dram_tensor("a_g", [NG, KT, BW_ * S], W16, kind="ExternalInput")
    b2_g = nc.dram_tensor("b2_g", [NG, KT, BW_ * S], W16, kind="ExternalInput")
    bt_g = nc.dram_tensor("bt_g", [NG, KT, BW_ * S], W16, kind="ExternalInput")
    a2_g = nc.dram_tensor("a2_g", [NG, KT, BW_ * S], W16, kind="ExternalInput")
    e_g = nc.dram_tensor("e_g", [NK // EB, KT, EB * D], W16,
                         kind="ExternalInput")
    out = nc.dram_tensor("out", [S, D], F32, kind="ExternalOutput")

    out_v = out.ap().rearrange("(k p) d -> p k d", p=KT)
    rg = [list(range(N_CORES))]

    with tile.TileContext(nc) as tc:
        with (
            tc.tile_pool(name="w", bufs=8) as wpool,
            tc.tile_pool(name="r", bufs=6) as rpool,
            tc.tile_pool(name="keep", bufs=1) as keep,
            tc.tile_pool(name="ps", bufs=8, space="PSUM") as pspool,
            tc.tile_pool(name="dram", bufs=1, space="DRAM") as dram,
        ):
            # partition-major bounce buffers: rank r's block in the gathered
            # output is rows [128r, 128r+128) = its full [128, 8*256] slice,
            # so in-writes and out-reads are single linear 0.5MB DMAs.
            cc_ins = [dram.tile([KT, SK * D], W16, name=f"cc_in_{i}",
                                tag=f"cci{i}") for i in range(3)]
            cc_outs = [dram.tile([KT * N_CORES, SK * D], W16,
                                 addr_space="Shared", name=f"cc_out_{i}",
                                 tag=f"cco{i}") for i in range(3)]

            # ---- embs preload: 4 x 1MB ----
            es = []
            for ge in range(NK // EB):
                er = rpool.tile([KT, EB * D], W16, name="er", tag="r")
                nc.sync.dma_start(er[:], e_g.ap()[ge])
                es.append(er)

            def stage(w_g, rhs_of, sink):
                """k-outer matmul stage: out_m += w[k,m].T @ rhs[k] over 64
                k-tiles, 8 m-tiles; sink(m, psum_tile) evicts each m-tile."""
                ps = [pspool.tile([KT, D], F32, name=f"ps_{m}", tag="ps")
                      for m in range(SK)]
                for g in range(NG):
                    w = wpool.tile([KT, BW_ * S], W16, name="w", tag="w")
                    nc.sync.dma_start(w[:], w_g.ap()[g])
                    for kk in range(BW_):
                        k = g * BW_ + kk
                        rh = rhs_of(k)
                        for m in range(SK):
                            nc.tensor.matmul(
                                ps[m][:],
                                w[:, kk * S + m * KT:kk * S + (m + 1) * KT],
                                rh, start=(k == 0), stop=(k == NK - 1))
                for m in range(SK):
                    sink(m, ps[m])

            def gathered_rhs(cc_out):
                """Prefetch the gathered [8*128, 8*256] tensor as 8 linear
                0.5MB DMAs (one per rank block); k-tile k lives in block
                k//SK at columns (k%SK)*D."""
                blocks = []
                for r in range(N_CORES):
                    b = rpool.tile([KT, SK * D], W16, name="gr", tag="r")
                    nc.sync.dma_start(
                        b[:], cc_out[r * KT:(r + 1) * KT, :])
                    blocks.append(b)
                return lambda k: blocks[k // SK][:, (k % SK) * D:
                                                 (k % SK + 1) * D]

            def sbuf_sink(t_sb):
                return lambda m, ps: nc.vector.tensor_copy(
                    t_sb[:, m * D:(m + 1) * D], ps[:])

            # ---- S1: t1 = A[:,e_c].T @ embs ----
            t1 = keep.tile([KT, SK * D], W16)
            stage(a_g,
                  lambda k: es[k // EB][:, (k % EB) * D:(k % EB + 1) * D],
                  sbuf_sink(t1))
            nc.sync.dma_start(cc_ins[0][:], t1[:])
            nc.gpsimd.collective_compute(
                "AllGather", mybir.AluOpType.bypass, replica_groups=rg,
                ins=[cc_ins[0][:].opt()], outs=[cc_outs[0][:].opt()])

            # ---- S2: t2[n_c] = B[:,n_c].T @ t1_full ----
            t2 = keep.tile([KT, SK * D], W16, name="t2", tag="t2")
            stage(b2_g, gathered_rhs(cc_outs[0]), sbuf_sink(t2))
            nc.sync.dma_start(cc_ins[1][:], t2[:])
            nc.gpsimd.collective_compute(
                "AllGather", mybir.AluOpType.bypass, replica_groups=rg,
                ins=[cc_ins[1][:].opt()], outs=[cc_outs[1][:].opt()])

            # ---- S3: t3 = B[e_c,:] @ t2_full ----
            t3 = keep.tile([KT, SK * D], W16, name="t3", tag="t3")
            stage(bt_g, gathered_rhs(cc_outs[1]), sbuf_sink(t3))
            nc.sync.dma_start(cc_ins[2][:], t3[:])
            nc.gpsimd.collective_compute(
                "AllGather", mybir.AluOpType.bypass, replica_groups=rg,
                ins=[cc_ins[2][:].opt()], outs=[cc_outs[2][:].opt()])

            # ---- S4: out[n_c] = A[n_c,:] @ t3_full, LeakyReLU fused ----
            o = keep.tile([KT, SK * D], F32, name="o", tag="o")
            neg = keep.tile([KT, D], F32, name="neg", tag="neg")

            def leaky_sink(m, ps):
                nc.vector.tensor_scalar_mul(neg[:], ps[:], LEAKY)
                nc.vector.tensor_max(o[:, m * D:(m + 1) * D], ps[:], neg[:])

            stage(a2_g, gathered_rhs(cc_outs[2]), leaky_sink)
            nc.sync.dma_start(out_v[:, :, :], o[:])

    nc.compile()
    return nc


def _fuse(t):
    """[NK,128,F] tile-major -> [NG,128,BW_*F] fused groups (linear DMA)."""
    nk, p, f = t.shape
    return np.ascontiguousarray(
        t.reshape(nk // BW_, BW_, p, f).transpose(0, 2, 1, 3)
    ).reshape(nk // BW_, p, BW_ * f)


def _fuse_e(eb):
    # [N, D] -> [NK/EB, 128, EB*D]
    return np.ascontiguousarray(
        eb.reshape(NK // EB, EB, KT, D).transpose(0, 2, 1, 3)
    ).reshape(NK // EB, KT, EB * D)


def _shard_inputs(inp_adj, att_adj, embs):
    A = np.asarray(att_adj, dtype=np.float32)   # [N, E]
    B = np.asarray(inp_adj, dtype=np.float32)   # [E, N]
    eb = np.asarray(embs, dtype=np.float32).astype(NP16)   # [N, D]
    e_gh = _fuse_e(eb)
    in_maps = []
    for c in range(N_CORES):
        s = slice(c * S, (c + 1) * S)
        a_col = A[:, s].astype(NP16)                       # [N, S] k=n
        b2_col = B[:, s].astype(NP16)                      # [E, S] k=e
        bt_col = np.ascontiguousarray(B[s, :].T).astype(NP16)   # [N, S] k=n
        a2_col = np.ascontiguousarray(A[s, :].T).astype(NP16)   # [E, S] k=e
        in_maps.append({
            "a_g": _fuse(a_col.reshape(NK, KT, S)),
            "b2_g": _fuse(b2_col.reshape(NK, KT, S)),
            "bt_g": _fuse(bt_col.reshape(NK, KT, S)),
            "a2_g": _fuse(a2_col.reshape(NK, KT, S)),
            "e_g": e_gh,
        })
    return in_maps


def _reset_device():
    """Recover wedged NeuronCores (NRT_EXEC_UNIT_UNRECOVERABLE) via axon."""
    import ctypes

    import jax
    try:
        jax.devices()
        lib = ctypes.CDLL("/opt/axon/libaxon_pjrt.so")
        lib.axon_reset.restype = ctypes.c_int64
        lib.axon_reset()
    except Exception:
        pass


def kernel(inp_adj, att_adj, embs, _trace=False):
    global _CACHED_NC
    if _CACHED_NC is None:
        _CACHED_NC = _build()
    nc = _CACHED_NC
    in_maps = _shard_inputs(inp_adj, att_adj, embs)
    try:
        res = run_bass_kernel_spmd(nc, in_maps,
                                   core_ids=list(range(N_CORES)),
                                   trace=_trace)
    except Exception:
        _reset_device()
        res = run_bass_kernel_spmd(nc, in_maps,
                                   core_ids=list(range(N_CORES)),
                                   trace=_trace)
    # core c owns out rows [c*S, (c+1)*S)
    full = np.empty((N, D), np.float32)
    for c in range(N_CORES):
        full[c * S:(c + 1) * S] = res.results[c]["out"]
    if _trace:
        kernel.last_exec_time_ns = res.exec_time_ns
    return full


# revision 7
# speedup vs baseline: 1.0662x; 1.0662x over previous
"""Distributed Trainium2 kernel for AttHGCNConv:
out = LeakyReLU_0.2( A @ B @ (B.T @ (A.T @ embs)) ),  A=att_adj [N,E], B=inp_adj [E,N].

Never materializes adj = A@B (~1.1 TFLOP); chains 4 thin matmuls of 34 GFLOP
each — memory-bound. 8-way sharded, ALL-GATHER ONLY dataflow (AG on 8 cores
~25us vs ~60us AllReduce):
  S1: t1_c = A[:,e_c].T @ embs        (A col-shard)   -> AG(t1)
  S2: t2[n_c] = B[:,n_c].T @ t1       (B col-shard)   -> AG(t2)
  S3: t3_c = B[e_c,:] @ t2            (B row-shard^T) -> AG(t3)
  S4: out[n_c] = A[n_c,:] @ t3        (A row-shard^T) + fused LeakyReLU,
     f32 rows stored directly; no final collective.

AG/compute pipelining via m-half splitting: each stage computes its output in
two m-halves and all-gathers each half as soon as it is done, while the other
half computes. Consumers k-consume in gathered-arrival order (half 0's tiles
first) via a host-side permutation of the lhsT k-tile layout. Pass order per
stage is (mh0,kh0),(mh1,kh0),(mh0,kh1),(mh1,kh1) so the rhs-half-1 wait is
covered by compute on rhs-half-0. fp16 operands (PSUM f32), fp16 wires.
Bounce buffers are partition-major so every wire-facing DMA is linear.
"""

import sys

for p in ("/opt/trn_rl_repo", "/root/.axon_site"):
    if p not in sys.path:
        sys.path.insert(0, p)

import numpy as np

import concourse.bass as bass  # noqa: F401
import concourse.mybir as mybir
import concourse.tile as tile
from concourse import bacc
from concourse.bass_utils import run_bass_kernel_spmd

N_CORES = 8
N = 8192  # nodes
E = 8192  # hyperedges
D = 256   # embedding dim
S = E // N_CORES   # 1024 per-core shard
S2 = S // 2        # m-half columns
KT = 128           # partition tile
NK = N // KT       # 64 k-tiles
HK = NK // 2       # 32 k-tiles per arrival half
SK = S // KT       # 8 m-tiles
SH = SK // 2       # 4 m-tiles per half
LEAKY = 0.2

BW_ = 4                      # k-tiles fused per weight DMA (0.5MB each)
NG = HK // BW_               # 8 weight DMAs per (mh, kh) pass
EB = 16                      # embs k-tiles per DMA

W16 = mybir.dt.float16       # matmul operand / wire dtype
F32 = mybir.dt.float32
NP16 = np.float16

_CACHED_NC = None

# weight DMA row count per stage: (mh, kh, g)
WROWS = 2 * 2 * NG


def _build():
    nc = bacc.Bacc("TRN2", target_bir_lowering=False, debug=False,
                   num_devices=N_CORES)

    a_g = nc.dram_tensor("a_g", [WROWS, KT, BW_ * S2], W16,
                         kind="ExternalInput")
    b2_g = nc.dram_tensor("b2_g", [WROWS, KT, BW_ * S2], W16,
                          kind="ExternalInput")
    bt_g = nc.dram_tensor("bt_g", [WROWS, KT, BW_ * S2], W16,
                          kind="ExternalInput")
    a2_g = nc.dram_tensor("a2_g", [WROWS, KT, BW_ * S2], W16,
                          kind="ExternalInput")
    e_g = nc.dram_tensor("e_g", [NK // EB, KT, EB * D], W16,
                         kind="ExternalInput")
    out = nc.dram_tensor("out", [S, D], F32, kind="ExternalOutput")

    out_v = out.ap().rearrange("(k p) d -> p k d", p=KT)
    rg = [list(range(N_CORES))]

    with tile.TileContext(nc) as tc:
        with (
            tc.tile_pool(name="w", bufs=10) as wpool,
            tc.tile_pool(name="e", bufs=4) as epool,
            tc.tile_pool(name="g", bufs=18) as gpool,
            tc.tile_pool(name="keep", bufs=1) as keep,
            tc.tile_pool(name="ps", bufs=8, space="PSUM") as pspool,
            tc.tile_pool(name="dram", bufs=1, space="DRAM") as dram,
        ):
            # per (stage-boundary, half) bounce buffers; partition-major so
            # rank r's block in the gathered output is rows [128r,128r+128)
            cc_ins = [[dram.tile([KT, SH * D], W16, name=f"cci_{i}_{h}",
                                 tag=f"cci{i}{h}") for h in range(2)]
                      for i in range(3)]
            cc_outs = [[dram.tile([KT * N_CORES, SH * D], W16,
                                  addr_space="Shared", name=f"cco_{i}_{h}",
                                  tag=f"cco{i}{h}") for h in range(2)]
                       for i in range(3)]

            # ---- embs preload: 4 x 1MB ----
            es = []
            for ge in range(NK // EB):
                er = epool.tile([KT, EB * D], W16, name="er", tag="e")
                nc.sync.dma_start(er[:], e_g.ap()[ge])
                es.append(er)

            def stage(w_g, rhs_of, sink):
                """Pass order (mh0,kh0),(mh1,kh0),(mh0,kh1),(mh1,kh1).
                rhs_of(t) maps consumption index t (0..63, kh-major) to an
                SBUF [128,256] slice. sink(hm, m, ps) evicts m-tile m of
                half hm; called for hm's 4 m-tiles after its last pass."""
                ps = [pspool.tile([KT, D], F32, name=f"ps_{m}", tag="ps")
                      for m in range(SK)]
                for hk in range(2):
                    for hm in range(2):
                        for g in range(NG):
                            row = (hm * 2 + hk) * NG + g
                            w = wpool.tile([KT, BW_ * S2], W16, name="w",
                                           tag="w")
                            nc.sync.dma_start(w[:], w_g.ap()[row])
                            for kk in range(BW_):
                                t = hk * HK + g * BW_ + kk
                                rh = rhs_of(t)
                                for m in range(SH):
                                    nc.tensor.matmul(
                                        ps[hm * SH + m][:],
                                        w[:, kk * S2 + m * KT:
                                          kk * S2 + (m + 1) * KT],
                                        rh, start=(t == 0), stop=(t == NK - 1))
                        if hk == 1:  # half hm complete
                            for m in range(SH):
                                sink(hm, m, ps[hm * SH + m])

            def gathered_rhs(bidx):
                """rhs from AG boundary bidx: 16 linear 0.25MB block DMAs,
                gated per arrival-half by the AG completing."""
                blocks = [[None] * N_CORES, [None] * N_CORES]

                def rhs(t):
                    hk, rem = divmod(t, HK)
                    r, j = divmod(rem, BW_)
                    if blocks[hk][r] is None:
                        b = gpool.tile([KT, SH * D], W16, name="gr", tag="g")
                        nc.sync.dma_start(
                            b[:], cc_outs[bidx][hk][r * KT:(r + 1) * KT, :])
                        blocks[hk][r] = b
                    return blocks[hk][r][:, j * D:(j + 1) * D]
                return rhs

            def ag_sink(bidx, t_sb):
                def sink(hm, m, ps):
                    dst = t_sb[:, (hm * SH + m) * D:(hm * SH + m + 1) * D]
                    if m % 2 == 0:
                        nc.vector.tensor_copy(dst, ps[:])
                    else:
                        nc.scalar.copy(dst, ps[:])
                    if m == SH - 1:
                        nc.sync.dma_start(
                            cc_ins[bidx][hm][:],
                            t_sb[:, hm * SH * D:(hm + 1) * SH * D])
                        nc.gpsimd.collective_compute(
                            "AllGather", mybir.AluOpType.bypass,
                            replica_groups=rg,
                            ins=[cc_ins[bidx][hm][:].opt()],
                            outs=[cc_outs[bidx][hm][:].opt()])
                return sink

            # ---- S1: t1 = A[:,e_c].T @ embs ----
            t1 = keep.tile([KT, SK * D], W16, name="t1", tag="t1")
            stage(a_g,
                  lambda t: es[t // EB][:, (t % EB) * D:(t % EB + 1) * D],
                  ag_sink(0, t1))

            # ---- S2: t2[n_c] = B[:,n_c].T @ t1_full ----
            t2 = keep.tile([KT, SK * D], W16, name="t2", tag="t2")
            stage(b2_g, gathered_rhs(0), ag_sink(1, t2))

            # ---- S3: t3 = B[e_c,:] @ t2_full ----
            t3 = keep.tile([KT, SK * D], W16, name="t3", tag="t3")
            stage(bt_g, gathered_rhs(1), ag_sink(2, t3))

            # ---- S4: out[n_c] = A[n_c,:] @ t3_full, LeakyReLU fused ----
            o = keep.tile([KT, SK * D], F32, name="o", tag="o")
            negs = [keep.tile([KT, D], F32, name=f"neg{h}", tag=f"neg{h}")
                    for h in range(2)]

            def leaky_sink(hm, m, ps):
                gm = hm * SH + m
                nc.vector.tensor_scalar_mul(negs[hm][:], ps[:], LEAKY)
                nc.vector.tensor_max(
                    o[:, gm * D:(gm + 1) * D], ps[:], negs[hm][:])
                if m == SH - 1:
                    nc.sync.dma_start(
                        out_v[:, hm * SH:(hm + 1) * SH, :],
                        o[:, hm * SH * D:(hm + 1) * SH * D])

            stage(a2_g, gathered_rhs(2), leaky_sink)

    nc.compile()
    return nc


def _relay(w, perm):
    """lhsT [8192, 1024] (k-rows, m-cols) -> [WROWS, KT, BW_*S2] with k-tiles
    in consumption order `perm` and rows ordered (mh, kh, g)."""
    wt = w.reshape(NK, KT, S)[perm]                    # [64, 128, 1024]
    wt = wt.reshape(2, NG, BW_, KT, 2, S2)             # hk, g, kk, p, hm, s2
    wt = wt.transpose(4, 0, 1, 3, 2, 5)                # hm, hk, g, p, kk, s2
    return np.ascontiguousarray(wt).reshape(WROWS, KT, BW_ * S2)


# consumption order for gathered rhs: t = (hk, r, j) -> k_global = r*SK+hk*SH+j
_PERM_G = np.array([r * SK + hk * BW_ + j
                    for hk in range(2) for r in range(N_CORES)
                    for j in range(BW_)])
_PERM_ID = np.arange(NK)


def _fuse_e(eb):
    # [N, D] -> [NK/EB, 128, EB*D]
    return np.ascontiguousarray(
        eb.reshape(NK // EB, EB, KT, D).transpose(0, 2, 1, 3)
    ).reshape(NK // EB, KT, EB * D)


def _shard_inputs(inp_adj, att_adj, embs):
    A = np.asarray(att_adj, dtype=np.float32)   # [N, E]
    B = np.asarray(inp_adj, dtype=np.float32)   # [E, N]
    eb = np.asarray(embs, dtype=np.float32).astype(NP16)   # [N, D]
    e_gh = _fuse_e(eb)
    in_maps = []
    for c in range(N_CORES):
        s = slice(c * S, (c + 1) * S)
        a_col = A[:, s].astype(NP16)                       # [N, S] k=n
        b2_col = B[:, s].astype(NP16)                      # [E, S] k=e
        bt_col = np.ascontiguousarray(B[s, :].T).astype(NP16)   # [N, S] k=n
        a2_col = np.ascontiguousarray(A[s, :].T).astype(NP16)   # [E, S] k=e
        in_maps.append({
            "a_g": _relay(a_col, _PERM_ID),
            "b2_g": _relay(b2_col, _PERM_G),
            "bt_g": _relay(bt_col, _PERM_G),
            "a2_g": _relay(a2_col, _PERM_G),
            "e_g": e_gh,
        })
    return in_maps


def _reset_device():
    """Recover wedged NeuronCores (NRT_EXEC_UNIT_UNRECOVERABLE) via axon."""
    import ctypes

    import jax
    try:
        jax.devices()
        lib = ctypes.CDLL("/opt/axon/libaxon_pjrt.so")
        lib.axon_reset.restype = ctypes.c_int64
        lib.axon_reset()
    except Exception:
        pass


def kernel(inp_adj, att_adj, embs, _trace=False):
    global _CACHED_NC
    if _CACHED_NC is None:
        _CACHED_NC = _build()
    nc = _CACHED_NC
    in_maps = _shard_inputs(inp_adj, att_adj, embs)
    try:
        res = run_bass_kernel_spmd(nc, in_maps,
                                   core_ids=list(range(N_CORES)),
                                   trace=_trace)
    except Exception:
        _reset_device()
        res = run_bass_kernel_spmd(nc, in_maps,
                                   core_ids=list(range(N_CORES)),
                                   trace=_trace)
    # core c owns out rows [c*S, (c+1)*S)
    full = np.empty((N, D), np.float32)
    for c in range(N_CORES):
        full[c * S:(c + 1) * S] = res.results[c]["out"]
    if _trace:
        kernel.last_exec_time_ns = res.exec_time_ns
    return full
